# revision 2
# baseline (speedup 1.0000x reference)
"""Trainium2 Bass kernel for nn_Aggregator (GNN message passing).

v6: the whole neighbor reduction rides the PE in fp8 DoubleRow perf mode.
Each DoubleRow matmul contracts TWO 128-row k-tiles (two neighbor chunks)
per pass at fp8 double-pump rate, so the 25-chunk sum is 12 paired matmuls
plus one plain fp8 matmul, all accumulating in fp32 PSUM. The v5 DVE tree
(the previous bottleneck after DMA: 95us busy, fp8 first level at 1x) is
gone entirely; the DVE stays idle and PE time drops well below the DMA
floor, which is now the only roofline:
  32.8MB xn(fp8) + 2.6MB xs(bf16) + 5.2MB out(bf16) = 40.6MB/core
  at ~360GB/s => ~113us.

DoubleRow requires fp8 weights. wn is quantized at x64 scale (wn sigma
~0.088; unscaled it would land in the e4m3 denormal range and lose ~30%
relative precision) and the 1/(64*25) is folded into the ACT scale of the
bias+relu epilogue. x_self stays bf16: fp8 x_self alone would contribute
~3.7% rel-to-max error, over the 2e-2 gate.

Layout per core (20 groups of 4 row-blocks): xnt [F, 25*512] fp8 per group
row, chunk n at columns [n*512,(n+1)*512); ring A DMA carries chunks 0..13
(7 pairs), ring B chunks 14..24 + xs. Stores ride the gpsimd SWDGE queue so
the HWDGE rings only ever carry loads.
"""

import sys

for _p in ("/opt/trn_rl_repo", "/root/.axon_site/_ro/trn_rl_repo"):
    if _p not in sys.path:
        sys.path.append(_p)

import numpy as np

from concourse import bacc, bass, mybir
from concourse.bass_utils import run_bass_kernel_spmd
from concourse.tile import TileContext

N_CORES = 8
B, H, NN, F = 8192, 10, 25, 128
D = 256
B_LOC = B // N_CORES          # 1024
R_LOC = B_LOC * H             # 10240 rows per core
P = 128
N_BLOCKS = R_LOC // P         # 80
G = 4                         # row-blocks per group
GP = G * P                    # 512 moving columns
N_GROUPS = N_BLOCKS // G      # 20
FP32 = mybir.dt.float32
BF16 = mybir.dt.bfloat16
FP8 = mybir.dt.float8e4
RELU = mybir.ActivationFunctionType.Relu
DROW = mybir.MatmulPerfMode.DoubleRow

CN = NN * GP                  # 12800 xnt columns per group row
CWB = F + 2                   # bf16 consts: ws | bias_self col | bias_neigh col
RA = 14                       # ring A chunks (7 DoubleRow pairs); ring B: 11
S_WN = 64.0                   # wn fp8 quantization scale (power of 2)
N_PAIRS = 12                  # DoubleRow pairs; chunk 24 rides a plain matmul


def build_bass(loop_iters=None, xn_bufs=6, xs_bufs=4, osb_bufs=3, psn_bufs=4,
               ra=RA, unroll_reps=1):
    nc = bacc.Bacc(None)
    xst = nc.dram_tensor("xst", [N_GROUPS * F, GP], BF16, kind="ExternalInput")
    xnt = nc.dram_tensor("xnt", [N_GROUPS * F, CN], FP8, kind="ExternalInput")
    cb = nc.dram_tensor("cb", [P, CWB], BF16, kind="ExternalInput")
    c8 = nc.dram_tensor("c8", [P, 2 * F], FP8, kind="ExternalInput")
    out = nc.dram_tensor("out", [N_GROUPS * P, 2 * GP], BF16, kind="ExternalOutput")

    with TileContext(nc) as tc:
        with (
            tc.tile_pool(name="const", bufs=1) as cpool,
            tc.tile_pool(name="xn", bufs=xn_bufs) as xnpool,
            tc.tile_pool(name="xs", bufs=xs_bufs) as xspool,
            tc.tile_pool(name="osb", bufs=osb_bufs) as opool,
            tc.tile_pool(name="psS", bufs=2, space="PSUM") as pspool_s,
            tc.tile_pool(name="psN", bufs=psn_bufs, space="PSUM") as pspool_n,
        ):
            cb_t = cpool.tile([P, CWB], BF16)
            nc.sync.dma_start(out=cb_t, in_=cb[:, :])
            c8_t = cpool.tile([P, 2 * F], FP8)
            nc.sync.dma_start(out=c8_t, in_=c8[:, :])
            ws_ap = cb_t[:, 0:F]
            bias_s_ap = cb_t[:, F : F + 1]
            bias_n_ap = cb_t[:, F + 1 : F + 2]
            wn2_ap = c8_t[:, :].rearrange("p (two f) -> p two f", two=2)
            wn1_ap = c8_t[:, 0:F]

            # Const loads stay OUTSIDE the hardware loop: reloading per
            # iteration puts a WAR-blocked DMA at the ring A queue head and
            # drains the pipeline at every loop boundary.
            if loop_iters is not None:
                loop_cm = tc.For_i(0, loop_iters, 1)
                loop_cm.__enter__()
            for _rep in range(unroll_reps):
                for g in range(N_GROUPS):
                    f0 = g * F
                    xn_t = xnpool.tile([P, CN], FP8)
                    # Ring A: chunks 0..ra-1. Ring B: xs, then chunks ra..24.
                    nc.sync.dma_start(
                        out=xn_t[:, 0 : ra * GP], in_=xnt[f0 : f0 + F, 0 : ra * GP]
                    )
                    xs_t = xspool.tile([P, GP], BF16)
                    nc.scalar.dma_start(out=xs_t, in_=xst[f0 : f0 + F, :])
                    nc.scalar.dma_start(
                        out=xn_t[:, ra * GP :], in_=xnt[f0 : f0 + F, ra * GP :]
                    )

                    self_ps = pspool_s.tile([P, GP], FP32)
                    neigh_ps = pspool_n.tile([P, GP], FP32)

                    for i in range(N_PAIRS):
                        rhs = xn_t[:, 2 * i * GP : (2 * i + 2) * GP].rearrange(
                            "p (two c) -> p two c", two=2
                        )
                        nc.tensor.matmul(
                            out=neigh_ps, lhsT=wn2_ap, rhs=rhs,
                            start=(i == 0), stop=False, perf_mode=DROW,
                            skip_group_check=True,
                        )
                    nc.tensor.matmul(
                        out=neigh_ps, lhsT=wn1_ap,
                        rhs=xn_t[:, 2 * N_PAIRS * GP : NN * GP],
                        start=False, stop=True, skip_group_check=True,
                    )
                    nc.tensor.matmul(
                        out=self_ps, lhsT=ws_ap, rhs=xs_t,
                        start=True, stop=True, skip_group_check=True,
                    )

                    o_sb = opool.tile([P, 2 * GP], BF16)
                    nc.scalar.activation(
                        out=o_sb[:, 0:GP], in_=self_ps, func=RELU, bias=bias_s_ap
                    )
                    nc.scalar.activation(
                        out=o_sb[:, GP : 2 * GP], in_=neigh_ps, func=RELU,
                        bias=bias_n_ap, scale=1.0 / (S_WN * NN),
                    )
                    # Stores alone on the gpsimd SWDGE queue.
                    nc.gpsimd.dma_start(
                        out=out[g * P : (g + 1) * P, :], in_=o_sb
                    )

        if loop_iters is not None:
            loop_cm.__exit__(None, None, None)

    nc.compile()
    return nc


def prepare(x_self, x_neigh, w_neigh, w_self, bias):
    """Relayout FULL inputs -> global staged arrays, in dram_tensor
    declaration order (axis 0 concat over the 8 cores for sharded tensors;
    consts are returned per-core and must be replicated by the caller)."""
    import ml_dtypes

    ng = N_CORES * N_GROUPS
    xn8 = np.asarray(x_neigh).astype(ml_dtypes.float8_e4m3)
    xn8 = xn8.reshape(ng, G, P, NN, F)
    xnt = np.ascontiguousarray(xn8.transpose(0, 4, 3, 1, 2)).reshape(ng * F, CN)
    xsb = np.asarray(x_self).astype(ml_dtypes.bfloat16).reshape(ng, G, P, F)
    xst = np.ascontiguousarray(xsb.transpose(0, 3, 1, 2)).reshape(ng * F, GP)

    cb = np.zeros((P, CWB), dtype=np.float32)
    cb[:, 0:F] = np.asarray(w_self, dtype=np.float32)
    cb[:, F] = np.asarray(bias, dtype=np.float32)[0:P]
    cb[:, F + 1] = np.asarray(bias, dtype=np.float32)[P:D]
    cb = cb.astype(ml_dtypes.bfloat16)

    wn8 = (np.asarray(w_neigh, dtype=np.float32) * S_WN).astype(
        ml_dtypes.float8_e4m3
    )
    c8 = np.ascontiguousarray(np.concatenate([wn8, wn8], axis=1))
    return xst, xnt, cb, c8


_NC_CACHE = None


def kernel(x_self, x_neigh, w_neigh, w_self, bias):
    global _NC_CACHE
    if _NC_CACHE is None:
        _NC_CACHE = build_bass()
    nc = _NC_CACHE

    xst, xnt, cb, c8 = prepare(x_self, x_neigh, w_neigh, w_self, bias)
    rg = N_GROUPS * F
    in_maps = [
        {"xst": xst[c * rg : (c + 1) * rg], "xnt": xnt[c * rg : (c + 1) * rg],
         "cb": cb, "c8": c8}
        for c in range(N_CORES)
    ]

    res = run_bass_kernel_spmd(nc, in_maps, list(range(N_CORES)))
    out = np.concatenate([res.results[c]["out"] for c in range(N_CORES)], axis=0)
    # out[g*P + d, half*GP + j*P + r] -> full[(g*G+j)*P + r, h... ] -> (B,H,D)
    ng = N_CORES * N_GROUPS
    o = out.astype(np.float32).reshape(ng, P, 2, G, P)
    o = o.transpose(0, 3, 4, 2, 1).reshape(B, H, D)
    return o


# revision 3
# speedup vs baseline: 1.0963x; 1.0963x over previous
"""Trainium2 Bass kernel for nn_Aggregator (GNN message passing).

v7: super-group DMAs + fp8 DoubleRow PE + small DVE assist.

HW microbenchmarks (dense data, all 8 cores) showed the real constraints:
  - per-HWDGE-queue marginal load rate ~0.59 ns/B-per-partition (two queues
    together ~435 GB/s/core), but ~1us of fixed cost per DMA instruction;
    the v5/v6 per-group DMAs (~7KB/partition) ran at an effective
    ~261-292 GB/s, which WAS the kernel roofline (139-146us).
  - plain fp8 512-col matmul ~365ns, fp8 DoubleRow pair ~437ns (2 chunks),
    DVE tree ~455ns/chunk, ACT epilogue ~530ns, stores share the DMA bus
    at ~0.34 ns/B.

So v7 amortizes the DMA fixed cost: the host packs each ring's data for a
SUPER-GROUP of 4 row-groups contiguously, one DMA per ring per super-group
(~27KB/partition each). Ring A carries chunks 0..12 (6656B/group), ring B
chunks 13..24 plus the bf16 x_self bytes (7168B/group; the self matmul rhs
is a bitcast BF16 view of the fp8 tile). Per-group load cadence ~4.5us +
store bus share ~0.7us => ~5us/group, ~100us/rep target.

Compute fits under that cadence: PE runs 8 fp8 DoubleRow pairs (chunks
0..11 and 13..16) + plain chunk 12 + the DVE-reduced chunk + the bf16 self
matmul (~4.6us); the DVE tree-reduces chunks 17..24 (~4.1us). ACT fuses
scale (undoing the x64 fp8 weight quantization scale and the 1/25 mean) +
bias + relu. Stores ride the gpsimd SWDGE queue.

Numerics: fp8-e4m3 xn and wn (wn at x64 scale: unscaled wn/25 would sit in
the e4m3 denormal range), bf16 xs/ws, fp32 PSUM -> rel-to-max ~8e-3
(gate 2e-2). fp8 x_self would add ~3.7% rel-to-max on its own, so it
stays bf16.
"""

import sys

for _p in ("/opt/trn_rl_repo", "/root/.axon_site/_ro/trn_rl_repo"):
    if _p not in sys.path:
        sys.path.append(_p)

import numpy as np

from concourse import bacc, bass, mybir
from concourse.bass_utils import run_bass_kernel_spmd
from concourse.tile import TileContext

N_CORES = 8
B, H, NN, F = 8192, 10, 25, 128
D = 256
B_LOC = B // N_CORES          # 1024
R_LOC = B_LOC * H             # 10240 rows per core
P = 128
N_BLOCKS = R_LOC // P         # 80
G = 4                         # row-blocks per group
GP = G * P                    # 512 moving columns
N_GROUPS = N_BLOCKS // G      # 20
SG = 4                        # groups per super-group (per DMA)
N_SG = N_GROUPS // SG         # 5
FP32 = mybir.dt.float32
BF16 = mybir.dt.bfloat16
FP8 = mybir.dt.float8e4
RELU = mybir.ActivationFunctionType.Relu
DROW = mybir.MatmulPerfMode.DoubleRow

CN = NN * GP                  # 12800 xn bytes per group row
CWB = F + 2                   # bf16 consts: ws | bias_self | bias_neigh
RA = 13                       # ring A chunks per group
ABYT = RA * GP                # 6656 ring A bytes/group/partition
BBYT = (NN - RA) * GP + 2 * GP  # 7168: chunks 13..24 + xs bf16 bytes
S_WN = 64.0                   # wn fp8 quantization scale (power of 2)
N_DVE = 8                     # chunks 17..24 reduced on the DVE


def build_bass(loop_iters=None, xt_bufs=3, osb_bufs=3, psn_bufs=4,
               red_bufs=3, n_dve=N_DVE, unroll_reps=1):
    nc = bacc.Bacc(None)
    xa = nc.dram_tensor("xa", [N_SG * F, SG * ABYT], FP8, kind="ExternalInput")
    xb = nc.dram_tensor("xb", [N_SG * F, SG * BBYT], FP8, kind="ExternalInput")
    cb = nc.dram_tensor("cb", [P, CWB], BF16, kind="ExternalInput")
    c8 = nc.dram_tensor("c8", [P, 2 * F], FP8, kind="ExternalInput")
    out = nc.dram_tensor("out", [N_GROUPS * P, 2 * GP], BF16,
                         kind="ExternalOutput")

    assert n_dve % 4 == 0 and 4 <= n_dve <= 12
    n_bpair = (NN - RA - n_dve) // 2       # DoubleRow pairs in ring B

    with TileContext(nc) as tc:
        with (
            tc.tile_pool(name="const", bufs=1) as cpool,
            tc.tile_pool(name="xt", bufs=xt_bufs) as xtpool,
            tc.tile_pool(name="red", bufs=red_bufs) as rpool,
            tc.tile_pool(name="osb", bufs=osb_bufs) as opool,
            tc.tile_pool(name="psS", bufs=2, space="PSUM") as pspool_s,
            tc.tile_pool(name="psN", bufs=psn_bufs, space="PSUM") as pspool_n,
        ):
            cb_t = cpool.tile([P, CWB], BF16)
            nc.sync.dma_start(out=cb_t, in_=cb[:, :])
            c8_t = cpool.tile([P, 2 * F], FP8)
            nc.sync.dma_start(out=c8_t, in_=c8[:, :])
            ws_ap = cb_t[:, 0:F]
            bias_s_ap = cb_t[:, F : F + 1]
            bias_n_ap = cb_t[:, F + 1 : F + 2]
            wn2_ap = c8_t[:, :].rearrange("p (two f) -> p two f", two=2)
            wn1_ap = c8_t[:, 0:F]

            # Const loads stay OUTSIDE the hardware loop (a WAR-blocked DMA
            # at a queue head would drain the pipeline at loop boundaries).
            if loop_iters is not None:
                loop_cm = tc.For_i(0, loop_iters, 1)
                loop_cm.__enter__()
            for _rep in range(unroll_reps):
                for sg in range(N_SG):
                    f0 = sg * F
                    A0 = 0
                    B0 = SG * ABYT
                    xt = xtpool.tile([P, SG * (ABYT + BBYT)], FP8)
                    nc.sync.dma_start(
                        out=xt[:, A0 : A0 + SG * ABYT], in_=xa[f0 : f0 + F, :]
                    )
                    nc.scalar.dma_start(
                        out=xt[:, B0 : B0 + SG * BBYT], in_=xb[f0 : f0 + F, :]
                    )

                    for gl in range(SG):
                        g = sg * SG + gl
                        a = A0 + gl * ABYT              # chunks 0..12
                        b = B0 + gl * BBYT              # chunks 13..24 | xs

                        def ck(n):
                            if n < RA:
                                return xt[:, a + n * GP : a + (n + 1) * GP]
                            return xt[:, b + (n - RA) * GP
                                      : b + (n - RA + 1) * GP]

                        def pair(n):
                            if n < RA:
                                o = a + n * GP
                            else:
                                o = b + (n - RA) * GP
                            return xt[:, o : o + 2 * GP].rearrange(
                                "p (two c) -> p two c", two=2
                            )

                        self_ps = pspool_s.tile([P, GP], FP32)
                        neigh_ps = pspool_n.tile([P, GP], FP32)

                        # PE: 6 ring-A pairs, plain chunk 12, ring-B pairs,
                        # then self (own PSUM), then the DVE-reduced chunk.
                        for k in range(6):
                            nc.tensor.matmul(
                                out=neigh_ps, lhsT=wn2_ap, rhs=pair(2 * k),
                                start=(k == 0), stop=False, perf_mode=DROW,
                                skip_group_check=True,
                            )
                        nc.tensor.matmul(
                            out=neigh_ps, lhsT=wn1_ap, rhs=ck(12),
                            start=False, stop=False, skip_group_check=True,
                        )
                        for k in range(n_bpair):
                            nc.tensor.matmul(
                                out=neigh_ps, lhsT=wn2_ap,
                                rhs=pair(RA + 2 * k),
                                start=False, stop=False, perf_mode=DROW,
                                skip_group_check=True,
                            )
                        # DVE: tree-reduce the last n_dve chunks in wide
                        # strips (fp8 first level, bf16 after).
                        dv0 = b + (NN - RA - n_dve) * GP
                        hw = n_dve // 2 * GP
                        red = rpool.tile([P, hw], BF16)
                        nc.vector.tensor_add(
                            out=red, in0=xt[:, dv0 : dv0 + hw],
                            in1=xt[:, dv0 + hw : dv0 + 2 * hw],
                        )
                        w = n_dve // 2
                        while w > 1:
                            h = w // 2
                            nc.vector.tensor_add(
                                out=red[:, 0 : h * GP], in0=red[:, 0 : h * GP],
                                in1=red[:, (w - h) * GP : w * GP],
                            )
                            w -= h

                        xs_ap = xt[:, b + (NN - RA) * GP
                                   : b + (NN - RA) * GP + 2 * GP].bitcast(BF16)
                        nc.tensor.matmul(
                            out=self_ps, lhsT=ws_ap, rhs=xs_ap,
                            start=True, stop=True, skip_group_check=True,
                        )
                        nc.tensor.matmul(
                            out=neigh_ps, lhsT=wn1_ap, rhs=red[:, 0:GP],
                            start=False, stop=True, skip_group_check=True,
                        )

                        o_sb = opool.tile([P, 2 * GP], BF16)
                        nc.scalar.activation(
                            out=o_sb[:, 0:GP], in_=self_ps, func=RELU,
                            bias=bias_s_ap,
                        )
                        nc.scalar.activation(
                            out=o_sb[:, GP : 2 * GP], in_=neigh_ps, func=RELU,
                            bias=bias_n_ap, scale=1.0 / (S_WN * NN),
                        )
                        # Stores alone on the gpsimd SWDGE queue.
                        nc.gpsimd.dma_start(
                            out=out[g * P : (g + 1) * P, :], in_=o_sb
                        )

        if loop_iters is not None:
            loop_cm.__exit__(None, None, None)

    nc.compile()
    return nc


def prepare(x_self, x_neigh, w_neigh, w_self, bias):
    """Relayout FULL inputs -> staged arrays in dram_tensor declaration
    order (xa, xb, cb, c8). xa/xb axis 0 concatenates the 8 cores; cb/c8
    are per-core (caller replicates)."""
    import ml_dtypes

    ng = N_CORES * N_GROUPS
    xn8 = np.asarray(x_neigh).astype(ml_dtypes.float8_e4m3)
    xn8 = xn8.reshape(ng, G, P, NN, F)
    # [g, f, n, j, r]: chunk n at bytes [n*GP, (n+1)*GP) of the group row
    xnt = np.ascontiguousarray(xn8.transpose(0, 4, 3, 1, 2)).reshape(
        ng, F, CN).view(np.uint8)
    xsb = np.asarray(x_self).astype(ml_dtypes.bfloat16).reshape(ng, G, P, F)
    xst = np.ascontiguousarray(xsb.transpose(0, 3, 1, 2)).reshape(
        ng, F, GP)
    xs_bytes = np.ascontiguousarray(xst).view(np.uint8)  # [ng, F, 1024]

    # Ring A: chunks 0..12. Ring B: chunks 13..24 + xs bytes.
    a = xnt[:, :, 0:ABYT]                                   # [ng, F, 6656]
    bpart = np.concatenate([xnt[:, :, ABYT:], xs_bytes], axis=2)  # 7168
    # [core, sg, gl, f, bytes] -> [core, sg, f, gl, bytes]
    xa = (a.reshape(N_CORES, N_SG, SG, F, ABYT)
          .transpose(0, 1, 3, 2, 4)
          .reshape(N_CORES * N_SG * F, SG * ABYT))
    xbv = (bpart.reshape(N_CORES, N_SG, SG, F, BBYT)
           .transpose(0, 1, 3, 2, 4)
           .reshape(N_CORES * N_SG * F, SG * BBYT))
    xa = np.ascontiguousarray(xa).view(ml_dtypes.float8_e4m3)
    xbv = np.ascontiguousarray(xbv).view(ml_dtypes.float8_e4m3)

    cb = np.zeros((P, CWB), dtype=np.float32)
    cb[:, 0:F] = np.asarray(w_self, dtype=np.float32)
    cb[:, F] = np.asarray(bias, dtype=np.float32)[0:P]
    cb[:, F + 1] = np.asarray(bias, dtype=np.float32)[P:D]
    cb = cb.astype(ml_dtypes.bfloat16)

    wn8 = (np.asarray(w_neigh, dtype=np.float32) * S_WN).astype(
        ml_dtypes.float8_e4m3
    )
    c8 = np.ascontiguousarray(np.concatenate([wn8, wn8], axis=1))
    return xa, xbv, cb, c8


_NC_CACHE = None


def kernel(x_self, x_neigh, w_neigh, w_self, bias):
    global _NC_CACHE
    if _NC_CACHE is None:
        _NC_CACHE = build_bass()
    nc = _NC_CACHE

    xa, xbv, cb, c8 = prepare(x_self, x_neigh, w_neigh, w_self, bias)
    rg = N_SG * F
    in_maps = [
        {"xa": xa[c * rg : (c + 1) * rg], "xb": xbv[c * rg : (c + 1) * rg],
         "cb": cb, "c8": c8}
        for c in range(N_CORES)
    ]

    res = run_bass_kernel_spmd(nc, in_maps, list(range(N_CORES)))
    out = np.concatenate([res.results[c]["out"] for c in range(N_CORES)],
                         axis=0)
    ng = N_CORES * N_GROUPS
    o = out.astype(np.float32).reshape(ng, P, 2, G, P)
    o = o.transpose(0, 3, 4, 2, 1).reshape(B, H, D)
    return o
